# revision 3
# baseline (speedup 1.0000x reference)
"""Trainium2 Bass kernel for nn_C3_layer (dense 5x5 VALID conv, 6->16 channels).

Full input x [32,6,512,512] f32 -> full output [32,16,508,508] f32.
Data-parallel over batch: 4 images per core across 8 NeuronCores.

v2: bf16 end-to-end on the device (tolerance is 2e-2; bf16 conv lands ~5e-3).
  - x converted to bf16 on host; input DMA traffic 15.3MB/core (was 25.4 f32)
  - matmuls in bf16 (1 col/cycle, same PE rate as fp32r) accumulating f32 PSUM
  - y written to HBM as bf16 (33MB/core, was 66) and upcast to f32 on host
  => HBM need drops from ~336GB/s (at the ~358 per-NC cap -> stalls + HAM
     cold-clock) to ~170GB/s, so the PE stays fed and warm.

Per-core decomposition (block-Toeplitz conv-as-matmul), 16-row superblocks:
  - one input DMA per superblock: xt [120, 512] holds 20 input rows
    (partition p = i*6 + ci, h-major); two 8-row blocks A (xt[0:72]) and
    B (xt[48:120]) share it. No SBUF->SBUF halo chain: input tiles are
    fully independent -> deep prefetch, one 120KB DMA per 16 output rows.
  - per block: one PSUM bank, 5 matmuls (one per kw tap)
      psum[m = co*R+r, ow] += sum_k T_kw[k, m] * xt[k, kw+ow]
    with T_kw[i*6+ci, co*R+r] = Wfull[co, ci, i-r, kw] (banded
    block-Toeplitz, built host-side, replicated to all cores)
  - PSUM evacuation + bias add split across engines: block A on VectorE
    (tensor_scalar_add), block B on ScalarE (activation Identity + bias),
    so neither engine is the serial bottleneck; out-DMAs issued by ScalarE
    (second HWDGE ring; SP's ring stays dedicated to input prefetch).
"""

import os

import numpy as np

KK = 5   # conv kernel size
R = 8    # output rows per full block
RT = 4   # tail block rows (508 = 31*16 + 8 + 4)
B_PER_CORE = 4
N_CORES = 8
H = 512
W = 512
HO = H - 4
WO = W - 4

CH3 = np.array([[0, 1, 2], [1, 2, 3], [2, 3, 4], [3, 4, 5], [0, 4, 5], [0, 1, 5]])
CH4 = np.array([[0, 1, 2, 3], [1, 2, 3, 4], [2, 3, 4, 5], [0, 3, 4, 5], [0, 1, 4, 5],
                [0, 1, 2, 5], [0, 1, 3, 4], [1, 2, 4, 5], [0, 2, 3, 5]])

_MM_DTYPE = os.environ.get("CONV_MM_DTYPE", "bfloat16")  # bfloat16 | float32r

# stash of the last BassKernelResults (for test.py)
LAST_RESULTS = None


def _np_mm_dtype():
    if _MM_DTYPE == "bfloat16":
        import ml_dtypes
        return ml_dtypes.bfloat16
    return np.float32


def _build_full_kernel(w3, w4, w6):
    Wf = np.zeros((16, 6, KK, KK), dtype=np.float32)
    Wf[np.arange(6)[:, None], CH3] = w3
    Wf[(6 + np.arange(9))[:, None], CH4] = w4
    Wf[15] = w6[0]
    return Wf


def _build_toeplitz(Wf, Rb):
    """T [5, 6*(Rb+4), 16*Rb]: T[kw, i*6+ci, co*Rb+r] = Wf[co, ci, i-r, kw].

    k is h-major (i*6+ci) so a window shifted by 8 rows is just a
    partition-offset slice of the same tile; m is co-major (co*Rb+r) so
    each out-DMA writes 16 contiguous HBM runs of Rb rows."""
    rows = Rb + 4
    T = np.zeros((KK, 6 * rows, 16 * Rb), dtype=np.float32)
    for kw in range(KK):
        for r in range(Rb):
            for kh in range(KK):
                i = r + kh
                for ci in range(6):
                    T[kw, i * 6 + ci, r::Rb] = Wf[:, ci, kh, kw]
    return T


def _build_bass():
    import contextlib

    import concourse.bacc as bacc
    import concourse.mybir as mybir
    import concourse.tile as tile

    f32 = mybir.dt.float32
    mm_dt = getattr(mybir.dt, _MM_DTYPE)
    # benchmarking only: repeat the whole conv body L times inside the NEFF
    loop_n = int(os.environ.get("CONV_BENCH_LOOP", "1"))

    nc = bacc.Bacc(name="conv5x5")
    x = nc.dram_tensor("x", [B_PER_CORE, 6, H, W], mm_dt, kind="ExternalInput")
    t = nc.dram_tensor("t", [KK, 6 * (R + 4), 16 * R], mm_dt, kind="ExternalInput")
    t2 = nc.dram_tensor("t2", [KK, 6 * (RT + 4), 16 * RT], mm_dt,
                        kind="ExternalInput")
    bias = nc.dram_tensor("bias", [16 * R, 1], f32, kind="ExternalInput")
    bias2 = nc.dram_tensor("bias2", [16 * RT, 1], f32, kind="ExternalInput")
    y = nc.dram_tensor("y", [B_PER_CORE, 16, HO, WO], mm_dt, kind="ExternalOutput")

    with tile.TileContext(nc) as tc:
        with (
            tc.tile_pool(name="const", bufs=1) as const_pool,
            tc.tile_pool(name="xin", bufs=8) as in_pool,
            tc.tile_pool(name="yout", bufs=8) as out_pool,
            tc.tile_pool(name="psum", bufs=8, space="PSUM") as psum_pool,
        ):
            tw = const_pool.tile([6 * (R + 4), KK * 16 * R], mm_dt, name="tw")
            nc.sync.dma_start(out=tw[:, :], in_=t.rearrange("kw k m -> k kw m"))
            tw2 = const_pool.tile([6 * (RT + 4), KK * 16 * RT], mm_dt, name="tw2")
            nc.sync.dma_start(out=tw2[:, :], in_=t2.rearrange("kw k m -> k kw m"))
            bias_sb = const_pool.tile([16 * R, 1], f32, name="bias_sb")
            nc.sync.dma_start(out=bias_sb[:, :], in_=bias[:, :])
            bias2_sb = const_pool.tile([16 * RT, 1], f32, name="bias2_sb")
            nc.sync.dma_start(out=bias2_sb[:, :], in_=bias2[:, :])

            loop_cm = (tc.For_i(0, loop_n, 1) if loop_n > 1
                       else contextlib.nullcontext())
            with loop_cm:
                _emit_conv_body(nc, tc, mybir, x, y, (tw, tw2),
                                (bias_sb, bias2_sb), in_pool, out_pool,
                                psum_pool, mm_dt, f32)
    nc.finalize()
    return nc


def _emit_conv_body(nc, tc, mybir, x, y, tws, biases,
                    in_pool, out_pool, psum_pool, mm_dt, f32):
    tw_full, tw_tail = tws
    bias_full, bias_tail = biases
    Ident = mybir.ActivationFunctionType.Identity

    # 63 full 8-row blocks + one 4-row tail per image -> 508 output rows.
    # Each block has its own input tile (matmul operands must sit at
    # base_partition 0) so all input DMAs are independent -> deep prefetch.
    n_full, tail_rows = divmod(HO, R)   # 63, 4
    blocks = [(bi * R, R) for bi in range(n_full)]
    if tail_rows:
        blocks.append((n_full * R, tail_rows))
    for b in range(B_PER_CORE):
        for bi, (oh0, Rb) in enumerate(blocks):
            Kb = 6 * (Rb + 4)
            Mb = 16 * Rb
            twb = tw_full if Rb == R else tw_tail
            bsb = bias_full if Rb == R else bias_tail
            xt = in_pool.tile([6 * (R + 4), W], mm_dt, name="xt", tag="xt")
            nc.sync.dma_start(
                out=xt[:Kb, :],
                in_=x[b, :, oh0:oh0 + Rb + 4, :].rearrange("c h w -> h c w"),
            )
            ps = psum_pool.tile([16 * R, WO], f32, name="ps", tag="ps")
            for kw in range(KK):
                nc.tensor.matmul(
                    ps[:Mb, :],
                    twb[:Kb, kw * Mb: (kw + 1) * Mb],
                    xt[:Kb, kw:kw + WO],
                    start=(kw == 0),
                    stop=(kw == KK - 1),
                )
            ot = out_pool.tile([16 * R, WO], mm_dt, name="ot", tag="ot")
            # evac+bias alternates DVE / ACT so neither engine serializes
            # the PSUM drain.
            if bi % 2 == 0:
                nc.vector.tensor_scalar_add(ot[:Mb, :], ps[:Mb, :],
                                            bsb[:Mb, :])
            else:
                nc.scalar.activation(ot[:Mb, :], ps[:Mb, :], Ident,
                                     bias=bsb[:Mb, :])
            # partition m = co*Rb + r is the natural (c, r, w) DRAM
            # order: per co, the Rb output rows are one contiguous run.
            nc.scalar.dma_start(
                out=y[b, :, oh0:oh0 + Rb, :],
                in_=ot[:Mb, :],
            )


def build_in_maps(x, w3, b3, w4, b4, w6, b6):
    """Host-side prep shared by kernel() and bench.py: returns the per-core
    input dicts for run_bass_kernel_spmd."""
    np_dt = _np_mm_dtype()
    x = np.ascontiguousarray(np.asarray(x, dtype=np.float32)).astype(np_dt)
    Wf = _build_full_kernel(np.asarray(w3, dtype=np.float32),
                            np.asarray(w4, dtype=np.float32),
                            np.asarray(w6, dtype=np.float32))
    T = _build_toeplitz(Wf, R).astype(np_dt)
    T2 = _build_toeplitz(Wf, RT).astype(np_dt)
    bias16 = np.concatenate([np.asarray(b3, dtype=np.float32),
                             np.asarray(b4, dtype=np.float32),
                             np.asarray(b6, dtype=np.float32)])
    bias_col = np.ascontiguousarray(
        np.repeat(bias16, R)[:, None], dtype=np.float32)    # [co*R+r, 1]
    bias2_col = np.ascontiguousarray(
        np.repeat(bias16, RT)[:, None], dtype=np.float32)   # [co*RT+r, 1]
    return [
        {"x": x[i * B_PER_CORE:(i + 1) * B_PER_CORE], "t": T, "t2": T2,
         "bias": bias_col, "bias2": bias2_col}
        for i in range(N_CORES)
    ]


def kernel(x, w3, b3, w4, b4, w6, b6):
    global LAST_RESULTS
    from concourse.bass_utils import run_bass_kernel_spmd

    in_maps = build_in_maps(x, w3, b3, w4, b4, w6, b6)
    nc = _build_bass()
    res = run_bass_kernel_spmd(
        nc, in_maps, core_ids=list(range(N_CORES)),
        trace=bool(int(os.environ.get("CONV_TRACE", "0"))),
    )
    LAST_RESULTS = res
    out = np.concatenate([r["y"] for r in res.results], axis=0)
    return np.ascontiguousarray(out.astype(np.float32))


# revision 16
# speedup vs baseline: 1.1289x; 1.1289x over previous
"""Trainium2 Bass kernel for nn_C3_layer (dense 5x5 VALID conv, 6->16 channels).

Full input x [32,6,512,512] f32 -> full output [32,16,508,508] f32.
Data-parallel over batch: 4 images per core across 8 NeuronCores.

On this part the PE streams ~1 col/0.85ns regardless of dtype (measured), so
runtime = streamed matmul columns. The plain block-Toeplitz scheme needs
5 kw-tap streams per 8 output rows (0.625 streams/row, M=16*8=128 wall).

v3 cuts that to 0.5 streams/row with a 2-way kw fold:
  - 6-row blocks; input tile xt [120, 512] with lane (kwp, i, ci):
      xt[kwp*60 + i*6 + ci, c] = x[ci, oh0+i, c + kwp],   kwp in {0,1}
    The kwp=1 lanes are x shifted one column -- produced by a GPSIMD
    SBUF->SBUF copy (SWDGE path, otherwise idle) from the kwp=0 lanes the
    input DMA just landed.
  - 3 matmul streams per block instead of 5:
      taps {0,1}: lhsT TA0 [120,96], rhs xt[:, 0:508]
      taps {2,3}: lhsT TA1 [120,96], rhs xt[:, 2:510]
      tap  {4}:   lhsT T4  [ 60,96], rhs xt[:60, 4:512]
    accumulating one f32 PSUM bank [96=16co*6r, 508].
  - all matmuls in bf16 (tolerance 2e-2; bf16 conv lands ~3e-3), y written
    bf16 and upcast on host; PSUM evac+bias alternates DVE/ACT; out-DMAs on
    the ACT HWDGE ring, input prefetch on the SP ring.
"""

import os

import numpy as np

KK = 5   # conv kernel size
R = 6    # output rows per full block (508 = 84*6 + 4)
RT = 4   # tail block rows
B_PER_CORE = 4
N_CORES = 8
H = 512
W = 512
HO = H - 4
WO = W - 4

CH3 = np.array([[0, 1, 2], [1, 2, 3], [2, 3, 4], [3, 4, 5], [0, 4, 5], [0, 1, 5]])
CH4 = np.array([[0, 1, 2, 3], [1, 2, 3, 4], [2, 3, 4, 5], [0, 3, 4, 5], [0, 1, 4, 5],
                [0, 1, 2, 5], [0, 1, 3, 4], [1, 2, 4, 5], [0, 2, 3, 5]])

_MM_DTYPE = os.environ.get("CONV_MM_DTYPE", "bfloat16")  # bfloat16 | float32r

# stash of the last BassKernelResults (for test.py)
LAST_RESULTS = None


def _np_mm_dtype():
    if _MM_DTYPE == "bfloat16":
        import ml_dtypes
        return ml_dtypes.bfloat16
    return np.float32


def _build_full_kernel(w3, w4, w6):
    Wf = np.zeros((16, 6, KK, KK), dtype=np.float32)
    Wf[np.arange(6)[:, None], CH3] = w3
    Wf[(6 + np.arange(9))[:, None], CH4] = w4
    Wf[15] = w6[0]
    return Wf


def _build_toeplitz2(Wf, Rb):
    """kw-folded banded Toeplitz for Rb-row blocks.

    TA [2, 2*6*rows, 16*Rb]: group g covers taps {2g, 2g+1};
      TA[g, kwp*6*rows + i*6 + ci, co*Rb + r] = Wf[co, ci, i-r, 2g+kwp]
    T4 [6*rows, 16*Rb]: tap 4 (kwp=0 lanes only)."""
    rows = Rb + 4
    K2 = 2 * 6 * rows
    M = 16 * Rb
    TA = np.zeros((2, K2, M), np.float32)
    T4 = np.zeros((6 * rows, M), np.float32)
    for g in range(2):
        for kwp in range(2):
            kw = 2 * g + kwp
            for r in range(Rb):
                for kh in range(KK):
                    i = r + kh
                    for ci in range(6):
                        TA[g, kwp * 6 * rows + i * 6 + ci, r::Rb] = \
                            Wf[:, ci, kh, kw]
    for r in range(Rb):
        for kh in range(KK):
            i = r + kh
            for ci in range(6):
                T4[i * 6 + ci, r::Rb] = Wf[:, ci, kh, 4]
    return TA, T4


def _build_bass():
    import contextlib

    import concourse.bacc as bacc
    import concourse.mybir as mybir
    import concourse.tile as tile

    f32 = mybir.dt.float32
    mm_dt = getattr(mybir.dt, _MM_DTYPE)
    # benchmarking only: repeat the whole conv body L times inside the NEFF
    loop_n = int(os.environ.get("CONV_BENCH_LOOP", "1"))

    rows_f = R + 4           # 10
    rows_t = RT + 4          # 8
    K2f, K1f, Mf = 2 * 6 * rows_f, 6 * rows_f, 16 * R    # 120, 60, 96
    K2t, K1t, Mt = 2 * 6 * rows_t, 6 * rows_t, 16 * RT   # 96, 48, 64

    nc = bacc.Bacc(name="conv5x5")
    x = nc.dram_tensor("x", [B_PER_CORE, 6, H, W], mm_dt, kind="ExternalInput")
    ta = nc.dram_tensor("ta", [K2f, 2 * Mf], mm_dt, kind="ExternalInput")
    t4 = nc.dram_tensor("t4", [K1f, Mf], mm_dt, kind="ExternalInput")
    tat = nc.dram_tensor("tat", [K2t, 2 * Mt], mm_dt, kind="ExternalInput")
    t4t = nc.dram_tensor("t4t", [K1t, Mt], mm_dt, kind="ExternalInput")
    bias6 = nc.dram_tensor("bias6", [Mf, 1], f32, kind="ExternalInput")
    bias4 = nc.dram_tensor("bias4", [Mt, 1], f32, kind="ExternalInput")
    y = nc.dram_tensor("y", [B_PER_CORE, 16, HO, WO], mm_dt, kind="ExternalOutput")

    with tile.TileContext(nc) as tc:
        with (
            tc.tile_pool(name="const", bufs=1) as const_pool,
            tc.tile_pool(name="xin", bufs=10) as in_pool,
            tc.tile_pool(name="yout", bufs=10) as out_pool,
            tc.tile_pool(name="psum", bufs=8, space="PSUM") as psum_pool,
        ):
            twa = const_pool.tile([K2f, 2 * Mf], mm_dt, name="twa")
            nc.sync.dma_start(out=twa[:, :], in_=ta[:, :])
            tw4 = const_pool.tile([K1f, Mf], mm_dt, name="tw4")
            nc.sync.dma_start(out=tw4[:, :], in_=t4[:, :])
            twat = const_pool.tile([K2t, 2 * Mt], mm_dt, name="twat")
            nc.sync.dma_start(out=twat[:, :], in_=tat[:, :])
            tw4t = const_pool.tile([K1t, Mt], mm_dt, name="tw4t")
            nc.sync.dma_start(out=tw4t[:, :], in_=t4t[:, :])
            bias6_sb = const_pool.tile([Mf, 1], f32, name="bias6_sb")
            nc.sync.dma_start(out=bias6_sb[:, :], in_=bias6[:, :])
            bias4_sb = const_pool.tile([Mt, 1], f32, name="bias4_sb")
            nc.sync.dma_start(out=bias4_sb[:, :], in_=bias4[:, :])

            loop_cm = (tc.For_i(0, loop_n, 1) if loop_n > 1
                       else contextlib.nullcontext())
            with loop_cm:
                _emit_conv_body(nc, tc, mybir, x, y,
                                (twa, tw4, twat, tw4t),
                                (bias6_sb, bias4_sb),
                                in_pool, out_pool, psum_pool, mm_dt, f32)
    nc.finalize()
    return nc


def _emit_conv_body(nc, tc, mybir, x, y, tws, biases,
                    in_pool, out_pool, psum_pool, mm_dt, f32):
    twa, tw4, twat, tw4t = tws
    bias6_sb, bias4_sb = biases
    Ident = mybir.ActivationFunctionType.Identity
    # A/B attribution for benching:
    #   full | pe (in+MM) | dma (in+out) | in | mm (MM from dummy) | out
    parts = os.environ.get("CONV_PARTS", "full")
    do_in = parts in ("full", "pe", "dma", "in")
    do_mm = parts in ("full", "pe", "mm")
    do_evac = parts == "full"
    do_out = parts in ("full", "dma", "out")
    rows_f = R + 4
    dummy_ot = dummy_xt = None
    if do_out and not do_evac:
        dummy_ot = in_pool.tile([16 * R, WO], mm_dt, name="dummy_ot",
                                tag="dummy_ot")
        nc.vector.memset(dummy_ot[:, :].bitcast(mybir.dt.uint32), 0)
    if do_mm and not do_in:
        dummy_xt = in_pool.tile([2 * 6 * rows_f, W], mm_dt, name="dummy_xt",
                                tag="dummy_xt")
        nc.vector.memset(dummy_xt[:, :].bitcast(mybir.dt.uint32), 0)

    samew = os.environ.get("CONV_PROBE", "") == "samew"  # timing-only probe
    flight = int(os.environ.get("CONV_FLIGHT", "4"))

    def emit_in(b, oh0, rows, K1, K2):
        xt = in_pool.tile([2 * 6 * rows_f, W], mm_dt, name="xt", tag="xt")
        # kwp=0 lanes straight from HBM; kwp=1 lanes (x shifted one
        # column) via a GPSIMD SBUF->SBUF copy on the otherwise-idle
        # SWDGE path. Lane (kwp, i, ci) = x[b, ci, oh0+i, w + kwp].
        nc.sync.dma_start(
            out=xt[:K1, :],
            in_=x[b, :, oh0:oh0 + rows, :].rearrange("c h w -> h c w"),
        )
        nc.gpsimd.dma_start(out=xt[K1:K2, 0:W - 1], in_=xt[0:K1, 1:W])
        return xt

    def emit_evac_out(b, bi, oh0, Rb, Mb, ps, bsb):
        if do_evac:
            ot = out_pool.tile([16 * R, WO], mm_dt, name="ot", tag="ot")
            # evac+bias alternates DVE / ACT so neither engine
            # serializes the PSUM drain.
            if bi % 2 == 0:
                nc.vector.tensor_scalar_add(ot[:Mb, :], ps[:Mb, :],
                                            bsb[:Mb, :])
            else:
                nc.scalar.activation(ot[:Mb, :], ps[:Mb, :], Ident,
                                     bias=bsb[:Mb, :])
        elif do_out:
            ot = dummy_ot  # bench-only: DMA path without evac
        if do_out:
            # partition m = co*Rb + r is the natural (c, r, w) DRAM
            # order: per co, the Rb output rows are one contiguous run.
            nc.scalar.dma_start(out=y[b, :, oh0:oh0 + Rb, :],
                                in_=ot[:Mb, :])

    n_full, tail_rows = divmod(HO, R)   # 84, 4
    rows, K2, K1, Mb = R + 4, 2 * 6 * (R + 4), 6 * (R + 4), 16 * R
    # full blocks in tap-major flights: one LDWEIGHTS per tap group feeds
    # `flight` consecutive blocks' matmuls (weight-stationary reuse).
    for b in range(B_PER_CORE):
        for f0 in range(0, n_full, flight):
            fb = [(f0 + j) * R for j in range(min(flight, n_full - f0))]
            xts = [emit_in(b, oh0, rows, K1, K2) if do_in else dummy_xt
                   for oh0 in fb]
            if do_mm:
                pss = [psum_pool.tile([16 * R, WO], f32, name="ps", tag="ps")
                       for _ in fb]
                for g in range(2):
                    lhs = twa[:K2, 0:Mb] if samew else \
                        twa[:K2, g * Mb:(g + 1) * Mb]
                    for j in range(len(fb)):
                        nc.tensor.matmul(pss[j][:Mb, :], lhs,
                                         xts[j][:K2, 2 * g:2 * g + WO],
                                         start=(g == 0), stop=False)
                for j in range(len(fb)):
                    lhs = twa[:K1, 0:Mb] if samew else tw4[:K1, :Mb]
                    nc.tensor.matmul(pss[j][:Mb, :], lhs,
                                     xts[j][:K1, 4:4 + WO],
                                     start=False, stop=True)
            for j, oh0 in enumerate(fb):
                emit_evac_out(b, f0 + j, oh0, R, Mb,
                              pss[j] if do_mm else None, bias6_sb)
        if tail_rows:
            # 4-row tail block, plain order
            oh0 = n_full * R
            rows_tl = tail_rows + 4
            K2t, K1t, Mt = 2 * 6 * rows_tl, 6 * rows_tl, 16 * tail_rows
            xt = emit_in(b, oh0, rows_tl, K1t, K2t) if do_in else dummy_xt
            if do_mm:
                ps = psum_pool.tile([16 * R, WO], f32, name="ps", tag="ps")
                nc.tensor.matmul(ps[:Mt, :], twat[:K2t, 0:Mt],
                                 xt[:K2t, 0:WO], start=True, stop=False)
                nc.tensor.matmul(ps[:Mt, :], twat[:K2t, Mt:2 * Mt],
                                 xt[:K2t, 2:2 + WO], start=False, stop=False)
                nc.tensor.matmul(ps[:Mt, :], tw4t[:K1t, :Mt],
                                 xt[:K1t, 4:4 + WO], start=False, stop=True)
            emit_evac_out(b, n_full, oh0, tail_rows, Mt,
                          ps if do_mm else None, bias4_sb)


def build_in_maps(x, w3, b3, w4, b4, w6, b6):
    """Host-side prep shared by kernel() and bench.py: returns the per-core
    input dicts for run_bass_kernel_spmd."""
    np_dt = _np_mm_dtype()
    xp = np.ascontiguousarray(np.asarray(x, dtype=np.float32)).astype(np_dt)
    Wf = _build_full_kernel(np.asarray(w3, dtype=np.float32),
                            np.asarray(w4, dtype=np.float32),
                            np.asarray(w6, dtype=np.float32))
    TA, T4 = _build_toeplitz2(Wf, R)
    TAt, T4t = _build_toeplitz2(Wf, RT)
    # sbuf layout: group g's columns side by side -> [K2, 2*M]
    ta = np.ascontiguousarray(
        TA.transpose(1, 0, 2).reshape(TA.shape[1], -1)).astype(np_dt)
    tat = np.ascontiguousarray(
        TAt.transpose(1, 0, 2).reshape(TAt.shape[1], -1)).astype(np_dt)
    t4 = np.ascontiguousarray(T4).astype(np_dt)
    t4t = np.ascontiguousarray(T4t).astype(np_dt)
    bias16 = np.concatenate([np.asarray(b3, dtype=np.float32),
                             np.asarray(b4, dtype=np.float32),
                             np.asarray(b6, dtype=np.float32)])
    bias6_col = np.ascontiguousarray(
        np.repeat(bias16, R)[:, None], dtype=np.float32)    # [co*R+r, 1]
    bias4_col = np.ascontiguousarray(
        np.repeat(bias16, RT)[:, None], dtype=np.float32)   # [co*RT+r, 1]
    return [
        {"x": xp[i * B_PER_CORE:(i + 1) * B_PER_CORE], "ta": ta, "t4": t4,
         "tat": tat, "t4t": t4t, "bias6": bias6_col, "bias4": bias4_col}
        for i in range(N_CORES)
    ]


def kernel(x, w3, b3, w4, b4, w6, b6):
    global LAST_RESULTS
    from concourse.bass_utils import run_bass_kernel_spmd

    in_maps = build_in_maps(x, w3, b3, w4, b4, w6, b6)
    nc = _build_bass()
    res = run_bass_kernel_spmd(
        nc, in_maps, core_ids=list(range(N_CORES)),
        trace=bool(int(os.environ.get("CONV_TRACE", "0"))),
    )
    LAST_RESULTS = res
    out = np.concatenate([r["y"] for r in res.results], axis=0)
    return np.ascontiguousarray(out.astype(np.float32))


# revision 17
# speedup vs baseline: 1.2145x; 1.0759x over previous
"""Trainium2 Bass kernel for nn_C3_layer (dense 5x5 VALID conv, 6->16 channels).

Full input x [32,6,512,512] f32 -> full output [32,16,508,508] f32.
Data-parallel over batch: 4 images per core across 8 NeuronCores.

On this part the PE streams ~1 col/0.85ns regardless of dtype (measured), so
runtime = streamed matmul columns. The plain block-Toeplitz scheme needs
5 kw-tap streams per 8 output rows (0.625 streams/row, M=16*8=128 wall).

v3 cuts that to 0.5 streams/row with a 2-way kw fold:
  - 6-row blocks; input tile xt [120, 512] with lane (kwp, i, ci):
      xt[kwp*60 + i*6 + ci, c] = x[ci, oh0+i, c + kwp],   kwp in {0,1}
    The kwp=1 lanes are x shifted one column -- produced by a GPSIMD
    SBUF->SBUF copy (SWDGE path, otherwise idle) from the kwp=0 lanes the
    input DMA just landed.
  - 3 matmul streams per block instead of 5:
      taps {0,1}: lhsT TA0 [120,96], rhs xt[:, 0:508]
      taps {2,3}: lhsT TA1 [120,96], rhs xt[:, 2:510]
      tap  {4}:   lhsT T4  [ 60,96], rhs xt[:60, 4:512]
    accumulating one f32 PSUM bank [96=16co*6r, 508].
  - all matmuls in bf16 (tolerance 2e-2; bf16 conv lands ~3e-3), y written
    bf16 and upcast on host; PSUM evac+bias alternates DVE/ACT; out-DMAs on
    the ACT HWDGE ring, input prefetch on the SP ring.
"""

import os

import numpy as np

KK = 5   # conv kernel size
R = 6    # output rows per full block (508 = 84*6 + 4)
RT = 4   # tail block rows
B_PER_CORE = 4
N_CORES = 8
H = 512
W = 512
HO = H - 4
WO = W - 4

CH3 = np.array([[0, 1, 2], [1, 2, 3], [2, 3, 4], [3, 4, 5], [0, 4, 5], [0, 1, 5]])
CH4 = np.array([[0, 1, 2, 3], [1, 2, 3, 4], [2, 3, 4, 5], [0, 3, 4, 5], [0, 1, 4, 5],
                [0, 1, 2, 5], [0, 1, 3, 4], [1, 2, 4, 5], [0, 2, 3, 5]])

_MM_DTYPE = os.environ.get("CONV_MM_DTYPE", "bfloat16")  # bfloat16 | float32r

# stash of the last BassKernelResults (for test.py)
LAST_RESULTS = None


def _np_mm_dtype():
    if _MM_DTYPE == "bfloat16":
        import ml_dtypes
        return ml_dtypes.bfloat16
    return np.float32


def _build_full_kernel(w3, w4, w6):
    Wf = np.zeros((16, 6, KK, KK), dtype=np.float32)
    Wf[np.arange(6)[:, None], CH3] = w3
    Wf[(6 + np.arange(9))[:, None], CH4] = w4
    Wf[15] = w6[0]
    return Wf


def _build_toeplitz2(Wf, Rb):
    """kw-folded banded Toeplitz for Rb-row blocks.

    TA [2, 2*6*rows, 16*Rb]: group g covers taps {2g, 2g+1};
      TA[g, kwp*6*rows + i*6 + ci, co*Rb + r] = Wf[co, ci, i-r, 2g+kwp]
    T4 [6*rows, 16*Rb]: tap 4 (kwp=0 lanes only)."""
    rows = Rb + 4
    K2 = 2 * 6 * rows
    M = 16 * Rb
    TA = np.zeros((2, K2, M), np.float32)
    T4 = np.zeros((6 * rows, M), np.float32)
    for g in range(2):
        for kwp in range(2):
            kw = 2 * g + kwp
            for r in range(Rb):
                for kh in range(KK):
                    i = r + kh
                    for ci in range(6):
                        TA[g, kwp * 6 * rows + i * 6 + ci, r::Rb] = \
                            Wf[:, ci, kh, kw]
    for r in range(Rb):
        for kh in range(KK):
            i = r + kh
            for ci in range(6):
                T4[i * 6 + ci, r::Rb] = Wf[:, ci, kh, 4]
    return TA, T4


def _build_bass():
    import contextlib

    import concourse.bacc as bacc
    import concourse.mybir as mybir
    import concourse.tile as tile

    f32 = mybir.dt.float32
    mm_dt = getattr(mybir.dt, _MM_DTYPE)
    # benchmarking only: repeat the whole conv body L times inside the NEFF
    loop_n = int(os.environ.get("CONV_BENCH_LOOP", "1"))

    rows_f = R + 4           # 10
    rows_t = RT + 4          # 8
    K2f, K1f, Mf = 2 * 6 * rows_f, 6 * rows_f, 16 * R    # 120, 60, 96
    K2t, K1t, Mt = 2 * 6 * rows_t, 6 * rows_t, 16 * RT   # 96, 48, 64

    nc = bacc.Bacc(name="conv5x5")
    x = nc.dram_tensor("x", [B_PER_CORE, 6, H, W], mm_dt, kind="ExternalInput")
    ta = nc.dram_tensor("ta", [K2f, 2 * Mf], mm_dt, kind="ExternalInput")
    t4 = nc.dram_tensor("t4", [K1f, Mf], mm_dt, kind="ExternalInput")
    tat = nc.dram_tensor("tat", [K2t, 2 * Mt], mm_dt, kind="ExternalInput")
    t4t = nc.dram_tensor("t4t", [K1t, Mt], mm_dt, kind="ExternalInput")
    bias6 = nc.dram_tensor("bias6", [Mf, 1], f32, kind="ExternalInput")
    bias4 = nc.dram_tensor("bias4", [Mt, 1], f32, kind="ExternalInput")
    y = nc.dram_tensor("y", [B_PER_CORE, 16, HO, WO], mm_dt, kind="ExternalOutput")

    with tile.TileContext(nc) as tc:
        with (
            tc.tile_pool(name="const", bufs=1) as const_pool,
            tc.tile_pool(name="xin", bufs=14) as in_pool,
            tc.tile_pool(name="yout", bufs=14) as out_pool,
            tc.tile_pool(name="psum", bufs=8, space="PSUM") as psum_pool,
        ):
            twa = const_pool.tile([K2f, 2 * Mf], mm_dt, name="twa")
            nc.sync.dma_start(out=twa[:, :], in_=ta[:, :])
            tw4 = const_pool.tile([K1f, Mf], mm_dt, name="tw4")
            nc.sync.dma_start(out=tw4[:, :], in_=t4[:, :])
            twat = const_pool.tile([K2t, 2 * Mt], mm_dt, name="twat")
            nc.sync.dma_start(out=twat[:, :], in_=tat[:, :])
            tw4t = const_pool.tile([K1t, Mt], mm_dt, name="tw4t")
            nc.sync.dma_start(out=tw4t[:, :], in_=t4t[:, :])
            bias6_sb = const_pool.tile([Mf, 1], f32, name="bias6_sb")
            nc.sync.dma_start(out=bias6_sb[:, :], in_=bias6[:, :])
            bias4_sb = const_pool.tile([Mt, 1], f32, name="bias4_sb")
            nc.sync.dma_start(out=bias4_sb[:, :], in_=bias4[:, :])

            loop_cm = (tc.For_i(0, loop_n, 1) if loop_n > 1
                       else contextlib.nullcontext())
            with loop_cm:
                _emit_conv_body(nc, tc, mybir, x, y,
                                (twa, tw4, twat, tw4t),
                                (bias6_sb, bias4_sb),
                                in_pool, out_pool, psum_pool, mm_dt, f32)
    nc.finalize()
    return nc


def _emit_conv_body(nc, tc, mybir, x, y, tws, biases,
                    in_pool, out_pool, psum_pool, mm_dt, f32):
    twa, tw4, twat, tw4t = tws
    bias6_sb, bias4_sb = biases
    Ident = mybir.ActivationFunctionType.Identity
    # A/B attribution for benching:
    #   full | pe (in+MM) | dma (in+out) | in | mm (MM from dummy) | out
    parts = os.environ.get("CONV_PARTS", "full")
    do_in = parts in ("full", "pe", "dma", "in")
    do_mm = parts in ("full", "pe", "mm")
    do_evac = parts == "full"
    do_out = parts in ("full", "dma", "out")
    rows_f = R + 4
    dummy_ot = dummy_xt = None
    if do_out and not do_evac:
        dummy_ot = in_pool.tile([16 * R, WO], mm_dt, name="dummy_ot",
                                tag="dummy_ot")
        nc.vector.memset(dummy_ot[:, :].bitcast(mybir.dt.uint32), 0)
    if do_mm and not do_in:
        dummy_xt = in_pool.tile([2 * 6 * rows_f, W], mm_dt, name="dummy_xt",
                                tag="dummy_xt")
        nc.vector.memset(dummy_xt[:, :].bitcast(mybir.dt.uint32), 0)

    samew = os.environ.get("CONV_PROBE", "") == "samew"  # timing-only probe
    flight = int(os.environ.get("CONV_FLIGHT", "4"))

    def emit_in(b, oh0, rows, K1, K2):
        xt = in_pool.tile([2 * 6 * rows_f, W], mm_dt, name="xt", tag="xt")
        # kwp=0 lanes straight from HBM; kwp=1 lanes (x shifted one
        # column) via a GPSIMD SBUF->SBUF copy on the otherwise-idle
        # SWDGE path. Lane (kwp, i, ci) = x[b, ci, oh0+i, w + kwp].
        nc.sync.dma_start(
            out=xt[:K1, :],
            in_=x[b, :, oh0:oh0 + rows, :].rearrange("c h w -> h c w"),
        )
        nc.gpsimd.dma_start(out=xt[K1:K2, 0:W - 1], in_=xt[0:K1, 1:W])
        return xt

    def emit_evac_out(b, bi, oh0, Rb, Mb, ps, bsb):
        if do_evac:
            ot = out_pool.tile([16 * R, WO], mm_dt, name="ot", tag="ot")
            # evac+bias alternates DVE / ACT so neither engine
            # serializes the PSUM drain.
            if bi % 2 == 0:
                nc.vector.tensor_scalar_add(ot[:Mb, :], ps[:Mb, :],
                                            bsb[:Mb, :])
            else:
                nc.scalar.activation(ot[:Mb, :], ps[:Mb, :], Ident,
                                     bias=bsb[:Mb, :])
        elif do_out:
            ot = dummy_ot  # bench-only: DMA path without evac
        if do_out:
            # partition m = co*Rb + r is the natural (c, r, w) DRAM
            # order: per co, the Rb output rows are one contiguous run.
            nc.scalar.dma_start(out=y[b, :, oh0:oh0 + Rb, :],
                                in_=ot[:Mb, :])

    n_full, tail_rows = divmod(HO, R)   # 84, 4
    rows, K2, K1, Mb = R + 4, 2 * 6 * (R + 4), 6 * (R + 4), 16 * R
    # full blocks in tap-major flights: one LDWEIGHTS per tap group feeds
    # `flight` consecutive blocks' matmuls (weight-stationary reuse).
    for b in range(B_PER_CORE):
        for f0 in range(0, n_full, flight):
            fb = [(f0 + j) * R for j in range(min(flight, n_full - f0))]
            xts = [emit_in(b, oh0, rows, K1, K2) if do_in else dummy_xt
                   for oh0 in fb]
            if do_mm:
                pss = [psum_pool.tile([16 * R, WO], f32, name="ps", tag="ps")
                       for _ in fb]
                for g in range(2):
                    lhs = twa[:K2, 0:Mb] if samew else \
                        twa[:K2, g * Mb:(g + 1) * Mb]
                    for j in range(len(fb)):
                        nc.tensor.matmul(pss[j][:Mb, :], lhs,
                                         xts[j][:K2, 2 * g:2 * g + WO],
                                         start=(g == 0), stop=False)
                for j in range(len(fb)):
                    lhs = twa[:K1, 0:Mb] if samew else tw4[:K1, :Mb]
                    nc.tensor.matmul(pss[j][:Mb, :], lhs,
                                     xts[j][:K1, 4:4 + WO],
                                     start=False, stop=True)
            for j, oh0 in enumerate(fb):
                emit_evac_out(b, f0 + j, oh0, R, Mb,
                              pss[j] if do_mm else None, bias6_sb)
        if tail_rows:
            # 4-row tail block, plain order
            oh0 = n_full * R
            rows_tl = tail_rows + 4
            K2t, K1t, Mt = 2 * 6 * rows_tl, 6 * rows_tl, 16 * tail_rows
            xt = emit_in(b, oh0, rows_tl, K1t, K2t) if do_in else dummy_xt
            if do_mm:
                ps = psum_pool.tile([16 * R, WO], f32, name="ps", tag="ps")
                nc.tensor.matmul(ps[:Mt, :], twat[:K2t, 0:Mt],
                                 xt[:K2t, 0:WO], start=True, stop=False)
                nc.tensor.matmul(ps[:Mt, :], twat[:K2t, Mt:2 * Mt],
                                 xt[:K2t, 2:2 + WO], start=False, stop=False)
                nc.tensor.matmul(ps[:Mt, :], tw4t[:K1t, :Mt],
                                 xt[:K1t, 4:4 + WO], start=False, stop=True)
            emit_evac_out(b, n_full, oh0, tail_rows, Mt,
                          ps if do_mm else None, bias4_sb)


def build_in_maps(x, w3, b3, w4, b4, w6, b6):
    """Host-side prep shared by kernel() and bench.py: returns the per-core
    input dicts for run_bass_kernel_spmd."""
    np_dt = _np_mm_dtype()
    xp = np.ascontiguousarray(np.asarray(x, dtype=np.float32)).astype(np_dt)
    Wf = _build_full_kernel(np.asarray(w3, dtype=np.float32),
                            np.asarray(w4, dtype=np.float32),
                            np.asarray(w6, dtype=np.float32))
    TA, T4 = _build_toeplitz2(Wf, R)
    TAt, T4t = _build_toeplitz2(Wf, RT)
    # sbuf layout: group g's columns side by side -> [K2, 2*M]
    ta = np.ascontiguousarray(
        TA.transpose(1, 0, 2).reshape(TA.shape[1], -1)).astype(np_dt)
    tat = np.ascontiguousarray(
        TAt.transpose(1, 0, 2).reshape(TAt.shape[1], -1)).astype(np_dt)
    t4 = np.ascontiguousarray(T4).astype(np_dt)
    t4t = np.ascontiguousarray(T4t).astype(np_dt)
    bias16 = np.concatenate([np.asarray(b3, dtype=np.float32),
                             np.asarray(b4, dtype=np.float32),
                             np.asarray(b6, dtype=np.float32)])
    bias6_col = np.ascontiguousarray(
        np.repeat(bias16, R)[:, None], dtype=np.float32)    # [co*R+r, 1]
    bias4_col = np.ascontiguousarray(
        np.repeat(bias16, RT)[:, None], dtype=np.float32)   # [co*RT+r, 1]
    return [
        {"x": xp[i * B_PER_CORE:(i + 1) * B_PER_CORE], "ta": ta, "t4": t4,
         "tat": tat, "t4t": t4t, "bias6": bias6_col, "bias4": bias4_col}
        for i in range(N_CORES)
    ]


def kernel(x, w3, b3, w4, b4, w6, b6):
    global LAST_RESULTS
    from concourse.bass_utils import run_bass_kernel_spmd

    in_maps = build_in_maps(x, w3, b3, w4, b4, w6, b6)
    nc = _build_bass()
    res = run_bass_kernel_spmd(
        nc, in_maps, core_ids=list(range(N_CORES)),
        trace=bool(int(os.environ.get("CONV_TRACE", "0"))),
    )
    LAST_RESULTS = res
    out = np.concatenate([r["y"] for r in res.results], axis=0)
    return np.ascontiguousarray(out.astype(np.float32))
